# revision 36
# baseline (speedup 1.0000x reference)
"""
Multi-head attention (b=4, n=1024, e=768, h=12, dh=64) on 8 trn2 NeuronCores.

Sharding: (batch, head-group) -> core.  Core c handles batch b=c//2 and head
group g=c%2 (6 of the 12 heads).  Each core computes QKV projection for its
heads, attention, and a row-parallel slice of the output projection, producing
a partial [1024, 768] output.  The host sums the two partials per batch
(the row-parallel all-reduce) during unsharding.

v2 layout/schedule (vs the v1 fp32r kernel):
- fp16 inputs.  x^T, Wq, Wk ride ONE packed dram tensor (6 chunk-major DMAs
  instead of 21) to cut both bytes and the ~620ns/issue sync-engine
  serialization.  Wv, Wp are single fp16 DMAs.
- 6 contraction chunks (768 rows, no bias row).  Q/K biases are added by DVE
  during the PSUM->SBUF copies (per-partition tensor_scalar); the V and proj
  biases are folded into a host-side constant row (exact: softmax rows sum
  to 1) added during unsharding.
- exp() output and V are bf16 (E reaches ~75, fp16 would overflow);
  Q/K/x/W are fp16 (same 10-bit mantissa the old fp32r path kept).
- The gpsimd library and the ACT exp table are prewarmed at kernel start
  (v1 lazy-loaded both mid-kernel, costing a ~6us pipeline stall).
- QKV-phase PE work is interleaved INTO the attention phase: E(head0) runs
  right after the p3=0 Q/K blocks finish, so the ACT engine (the 55us exp
  bottleneck) starts ~35us earlier than v1's phase-sequential schedule.
- Softmax denominators come from a ones-column in each V slot.  AV PSUM is
  split into qc-halves so the next head's AV can start as soon as the first
  half is drained.  Head 5's reciprocal broadcast runs on the PE (ones
  matmul) so the output projection starts right behind it.
"""

import math
from contextlib import ExitStack

import numpy as np

import concourse.mybir as mybir
import concourse.tile as tile
from concourse import bacc
from concourse.bass_utils import run_bass_kernel_spmd

EMB = 768
HEADS = 12
DH = 64
N = 1024
B = 4
HPC = 6  # heads per core
CC = 6  # contraction chunks (768 = 6*128)
XO = 0  # xT cols in IN
QO = 1024  # Wq cols
KO = 1408  # Wk cols
BO = 1792  # bias cols (chunk 0 rows): q-bias head 0..5 | pad2 | k-bias head 0..5
INW = BO + 16
VW = HPC * 65 + 63  # V slots overlap: lhsT reads 128 cols from slot h*65
F32 = mybir.dt.float32
F32R = mybir.dt.float32r
F16 = mybir.dt.float16
BF16 = mybir.dt.bfloat16

N_CORES = 8
DEBUG = False


def build_program():
    nc = bacc.Bacc("TRN2", target_bir_lowering=False, debug=False, num_devices=N_CORES)

    IN = nc.dram_tensor("IN", [CC * 128, INW], F16, kind="ExternalInput").ap()
    Wv = nc.dram_tensor("Wv", [CC * 128, HPC * 65], F16, kind="ExternalInput").ap()
    Wp = nc.dram_tensor("Wp", [3 * 128, EMB], F16, kind="ExternalInput").ap()
    out = nc.dram_tensor("out", [EMB, N], F16, kind="ExternalOutput").ap()  # out^T
    if DEBUG:
        dQT = nc.dram_tensor("dQT", [HPC * 128, N], F32, kind="ExternalOutput").ap()
        dKT = nc.dram_tensor("dKT", [HPC * 128, N], F32, kind="ExternalOutput").ap()
        dV = nc.dram_tensor("dV", [8 * 128, VW], F32, kind="ExternalOutput").ap()
        dOT = nc.dram_tensor("dOT", [3 * 128, N], F32, kind="ExternalOutput").ap()
        dEX = nc.dram_tensor("dEX", [128, N], F32, kind="ExternalOutput").ap()
        dOU = nc.dram_tensor("dOU", [128, N], F32, kind="ExternalOutput").ap()
        dPE = nc.dram_tensor("dPE", [128, N], F32, kind="ExternalOutput").ap()
        dRB = nc.dram_tensor("dRB", [128, N], F32, kind="ExternalOutput").ap()

    Exp = mybir.ActivationFunctionType.Exp

    with tile.TileContext(nc) as tc, ExitStack() as ctx:
        const = ctx.enter_context(tc.tile_pool(name="const", bufs=1))

        # ---- resident SBUF ------------------------------------------------
        IN_sb = const.tile([128, CC, INW], F16)
        Wv_sb = const.tile([128, CC, HPC * 65], F16)
        Wp_sb = const.tile([128, 3, EMB], F16)
        QT_sb = const.tile([128, HPC, N], F16)  # head h in parts 0..63, pad 64..127
        KT_sb = const.tile([128, HPC, N], F16)
        V_sb = const.tile([128, 8, VW], BF16)  # V natural, 8 token chunks
        OT_sb = const.tile([128, 3, N], F16)  # normalized O^T, head pair per chunk
        bias_sb = const.tile([128, 16], F32)
        warm = const.tile([128, 640], F16)  # junk matmul operand
        dummy = const.tile([128, 8], F32)  # prewarm sources
        dummy_a = const.tile([128, 8], F32)
        dummy_g = const.tile([128, 8], F32)
        ones64f = const.tile([1, 64], F32)
        ones64 = const.tile([1, 64], F32R)  # PE-broadcast stationary (head 5)
        if DEBUG:
            dbg_hold_ex = const.tile([128, N], F32)
            dbg_hold_pe = const.tile([128, N], F32)
            dbg_hold_ou = const.tile([128, N], F32)
            dbg_hold_rb = const.tile([128, N], F32)

        # prologue (engines otherwise idle during DMA bring-up).  DVE keeps
        # only what gates early work: warm (junk operand), the p3=0 head-slot
        # pads (E head 0/1 read them), V pad.  The p3=1,2 pads go to gpsimd,
        # whose queue is free until the head-0 broadcast (~35us in).
        nc.vector.memset(warm[:], 0.125)
        nc.vector.memset(dummy[:], 0.0)
        nc.vector.memset(ones64f[:], 1.0)
        nc.vector.tensor_copy(ones64[:], ones64f[:])
        nc.vector.memset(QT_sb[64:128, 0:2, :], 0.0)
        nc.vector.memset(KT_sb[64:128, 0:2, :], 0.0)
        nc.vector.memset(V_sb[:, :, HPC * 65 :], 0.0)

        # ---- input DMAs.  All 8 on the sync queue: transfers are
        # HBM-bound, so serialized issue order doubles as priority order
        # (the paced Q/K groups consume chunks in exactly this order).
        IN_r = IN.rearrange("(c p) n -> p c n", p=128)
        for c in range(CC):
            nc.sync.dma_start(IN_sb[:, c, :], IN_r[:, c, :])
        # The p3=1,2 pad memsets run first on gpsimd so the Wv/Wp issues
        # behind them only hit the wire at ~14us -- after the six IN chunks
        # (the critical path to the first exp) have drained the HBM.
        # Concurrent DMA queues share bandwidth round-robin.
        nc.gpsimd.memset(QT_sb[64:128, 2:6, :], 0.0)
        nc.gpsimd.memset(KT_sb[64:128, 2:6, :], 0.0)
        nc.gpsimd.dma_start(Wv_sb[:], Wv.rearrange("(c p) n -> p c n", p=128))
        nc.gpsimd.dma_start(Wp_sb[:], Wp.rearrange("(c p) n -> p c n", p=128))
        # ACT exp-table prewarm (table DMA ~2.7us) + gpsimd library prewarm
        nc.scalar.activation(dummy_a[0:1, :], dummy[0:1, :], Exp)
        nc.gpsimd.partition_broadcast(dummy_g[0:64, :], dummy[0:1, :])
        # bias cast (waits on IN chunk 0)
        nc.vector.tensor_copy(bias_sb[:], IN_sb[:, 0, BO : BO + 16])

        # ---- PSUM pools (16KB/partition total).  Stack order matters:
        # psA (QKV groups) releases mid-attention so a third exp buffer
        # (psB2) can take its banks; psB2+psB release before proj (psD).
        psO = ctx.enter_context(tc.tile_pool(name="psO", bufs=1, space="PSUM"))
        psB = tc.alloc_tile_pool(name="psB", bufs=1, space="PSUM")  # e 2x4KB
        psA = tc.alloc_tile_pool(name="psA", bufs=1, space="PSUM")  # mm 2x2KB

        expp = ctx.enter_context(tc.tile_pool(name="expp", bufs=13))
        oupool = ctx.enter_context(tc.tile_pool(name="oupool", bufs=3))
        rpool = ctx.enter_context(tc.tile_pool(name="rpool", bufs=2))

        # ---- emit helpers -------------------------------------------------
        def emit_junk(n, tag_tile):
            for _ in range(n):
                nc.tensor.matmul(
                    tag_tile[:, 0:512], warm[:, 0:128], warm[:, 128:640],
                    start=True, stop=True,
                )

        def qk_copy_a(g, qc, T_sb, p3, kbias):
            bb = 8 if kbias else 0
            nc.vector.tensor_scalar_add(
                out=T_sb[0:64, 2 * p3, qc * 512 : (qc + 1) * 512],
                in0=g[0:64, :],
                scalar1=bias_sb[0:64, bb + 2 * p3 : bb + 2 * p3 + 1],
            )

        def qk_copy_b(g, qc, T_sb, p3, kbias):
            bb = 8 if kbias else 0
            nc.vector.tensor_scalar_add(
                out=T_sb[0:64, 2 * p3 + 1, qc * 512 : (qc + 1) * 512],
                in0=g[64:128, :],
                scalar1=bias_sb[64:128, bb + 2 * p3 + 1 : bb + 2 * p3 + 2],
            )

        def qk_copy_b_act(g, qc, T_sb, p3, kbias):
            # odd-head-slot copy on the ACT engine (idle until the first
            # exp); Copy rides along in every activation table set
            bb = 8 if kbias else 0
            nc.scalar.activation(
                T_sb[0:64, 2 * p3 + 1, qc * 512 : (qc + 1) * 512],
                g[64:128, :],
                mybir.ActivationFunctionType.Identity,
                bias=bias_sb[64:128, bb + 2 * p3 + 1 : bb + 2 * p3 + 2],
            )

        def qk_copy(g, qc, T_sb, p3, kbias):
            """PSUM group [128ch, 512 tok] -> two head slots (+ bias)."""
            qk_copy_a(g, qc, T_sb, p3, kbias)
            qk_copy_b(g, qc, T_sb, p3, kbias)

        def emit_qk_half(g, p3, qc, wofs, cs):
            for c in cs:
                nc.tensor.matmul(
                    g[:],
                    IN_sb[:, c, wofs + p3 * 128 : wofs + (p3 + 1) * 128],
                    IN_sb[:, c, qc * 512 : (qc + 1) * 512],
                    start=(c == 0),
                    stop=(c == CC - 1),
                )

        def make_qk_filler(p3, qc, wofs, T_sb, kbias, name):
            # two half-passes per group so E chunks interleave twice as often
            st = {}

            def fa():
                st["g"] = psA.tile([128, 512], F32, tag="mm", bufs=2, name=name)
                emit_qk_half(st["g"], p3, qc, wofs, range(3))

            def fb():
                emit_qk_half(st["g"], p3, qc, wofs, range(3, CC))
                qk_copy(st["g"], qc, T_sb, p3, kbias)

            return [fa, fb]

        def make_v_filler(t8):
            st = {}

            def half(g, cs):
                for c in cs:
                    nc.tensor.matmul(
                        g[:, 0 : HPC * 65],
                        IN_sb[:, c, t8 * 128 : (t8 + 1) * 128],
                        Wv_sb[:, c, :],
                        start=(c == 0),
                        stop=(c == CC - 1),
                    )

            def fa():
                st["g"] = psA.tile([128, 512], F32, tag="mm", bufs=2, name=f"psv_{t8}")
                half(st["g"], range(3))

            def fb():
                g = st["g"]
                half(g, range(3, CC))
                nc.vector.tensor_copy(V_sb[:, t8, 0 : HPC * 65], g[:, 0 : HPC * 65])
                nc.vector.memset(V_sb[:, t8, DH : HPC * 65 : 65], 1.0)

            return [fa, fb]

        e_state = {"n": 0, "psB2": None}

        def emit_e(h, kc):
            # 3-deep exp pipeline once psB2 exists (decouples ACT from the
            # PE's E/AV iteration); 2-deep before that (fillers cover it)
            if e_state["psB2"] is not None and e_state["n"] % 3 == 2:
                pe = e_state["psB2"].tile(
                    [128, N], F32, tag="e3", bufs=1, name=f"pe_{h}_{kc}"
                )
            else:
                pe = psB.tile([128, N], F32, tag="e", bufs=2, name=f"pe_{h}_{kc}")
            e_state["n"] += 1
            for qc in range(2):
                nc.tensor.matmul(
                    pe[:, qc * 512 : (qc + 1) * 512],
                    KT_sb[:, h, kc * 128 : (kc + 1) * 128],
                    QT_sb[:, h, qc * 512 : (qc + 1) * 512],
                    start=True,
                    stop=True,
                )
            ex = expp.tile([128, N], BF16, tag="ex", name=f"ex_{h}_{kc}")
            nc.scalar.activation(ex[:], pe[:], Exp)
            if DEBUG and (h, kc) == (0, 0):
                nc.vector.tensor_copy(dbg_hold_ex[:], ex[:])
                nc.vector.tensor_copy(dbg_hold_pe[:], pe[:])
            return ex

        def begin_head(h):
            poa = psO.tile([65, 512], F32, tag="o2", bufs=2, name=f"poa_{h}")
            pob = psO.tile([65, 512], F32, tag="o2", bufs=2, name=f"pob_{h}")
            return (poa, pob)

        def emit_av(h, kc, po, ex):
            for qc in range(2):
                nc.tensor.matmul(
                    po[qc][0:65, :],
                    V_sb[:, kc, h * 65 : h * 65 + 65],
                    ex[:, qc * 512 : (qc + 1) * 512],
                    start=(kc == 0),
                    stop=(kc == 7),
                )

        def norm_chain(h, po, last=False):
            """po halves -> normalized OT chunk.  Row 64 of each half is the
            exp-sum; reciprocal needs a partition-0 copy first (HW quirk).
            The first PSUM half frees early so the next head's AV reuses
            it with minimal stall."""
            p3, half = divmod(h, 2)
            off = 64 * half
            ss = rpool.tile([1, N], F32, tag="ss", name=f"ss_{h}")
            ou = oupool.tile([65, N], F32, tag="ou", name=f"ou_{h}")
            rs = rpool.tile([1, N], F32, tag="rs", name=f"rs_{h}")
            if not last:
                nc.vector.tensor_copy(ss[:, 0:512], po[0][64:65, :])
                nc.vector.tensor_copy(ou[:, 0:512], po[0][0:65, :])
                nc.vector.tensor_copy(ss[:, 512:1024], po[1][64:65, :])
                nc.vector.tensor_copy(ou[:, 512:1024], po[1][0:65, :])
                nc.vector.reciprocal_approx_fast(rs[:], ss[:])
                rb = rpool.tile([64, N], F32, tag="rb", name=f"rb_{h}")
                nc.gpsimd.partition_broadcast(rb[:], rs[:])
                nc.vector.tensor_mul(
                    OT_sb[off : off + 64, p3, :], ou[0:64, :], rb[:]
                )
                if DEBUG and h == 0:
                    nc.vector.tensor_copy(dbg_hold_ou[0:65, :], ou[0:65, :])
                    nc.vector.tensor_copy(dbg_hold_rb[0:64, :], rb[:])
                return None
            # tail chain: the sums copies ride the (now idle) ACT engine in
            # parallel with the DVE ou copies, and everything is split by
            # qc halves so the first half's reciprocal/cast/broadcast land
            # early for proj's c2 passes on blocks 0-3
            rsr = rpool.tile([1, N], F32R, tag="rsr", name="rsr")
            nc.scalar.copy(ss[:, 0:512], po[0][64:65, :])
            nc.vector.reciprocal_approx_fast(rs[:, 0:512], ss[:, 0:512])
            nc.vector.tensor_copy(rsr[0:1, 0:512], rs[:, 0:512])
            nc.vector.tensor_copy(ou[:, 0:512], po[0][0:65, :])
            nc.scalar.copy(ss[:, 512:1024], po[1][64:65, :])
            nc.vector.reciprocal_approx_fast(rs[:, 512:1024], ss[:, 512:1024])
            nc.vector.tensor_copy(rsr[0:1, 512:1024], rs[:, 512:1024])
            nc.vector.tensor_copy(ou[:, 512:1024], po[1][0:65, :])
            return (ou, rsr, off, p3)

        # ---- phase 1a: junk + paced p3=0 Q/K groups -----------------------
        # Junk matmuls keep the PE's HAM activity window busy through the
        # DMA-paced stretch (an idle window declocks the PE to 1.2 GHz).
        # gQ0/gQ1 accumulate in the mm pool; gK0 borrows a psB exp slot so
        # all three p3=0 groups pace the incoming chunks concurrently.
        jt = psB.tile([128, N], F32, tag="e", bufs=2, name="junk")
        emit_junk(12, jt)
        gQ0 = psA.tile([128, 512], F32, tag="mm", bufs=2, name="gQ0")
        gQ1 = psA.tile([128, 512], F32, tag="mm", bufs=2, name="gQ1")
        gK0 = psB.tile([128, 512], F32, tag="e", bufs=2, name="gK0")
        for c in range(CC):
            nc.tensor.matmul(
                gQ0[:], IN_sb[:, c, QO : QO + 128], IN_sb[:, c, 0:512],
                start=(c == 0), stop=(c == CC - 1),
            )
            nc.tensor.matmul(
                gQ1[:], IN_sb[:, c, QO : QO + 128], IN_sb[:, c, 512:1024],
                start=(c == 0), stop=(c == CC - 1),
            )
            nc.tensor.matmul(
                gK0[:], IN_sb[:, c, KO : KO + 128], IN_sb[:, c, 0:512],
                start=(c == 0), stop=(c == CC - 1),
            )
            if c < CC - 1:
                emit_junk(3, jt)
        # head-0 halves (E(0,*) inputs) on DVE; head-1 halves on ACT,
        # which is idle until the first exp at ~17us
        qk_copy_a(gQ0, 0, QT_sb, 0, False)
        qk_copy_a(gQ1, 1, QT_sb, 0, False)
        qk_copy_a(gK0, 0, KT_sb, 0, True)
        qk_copy_b_act(gK0, 0, KT_sb, 0, True)
        qk_copy_b_act(gQ0, 0, QT_sb, 0, False)
        qk_copy_b_act(gQ1, 1, QT_sb, 0, False)


        # ---- phase 1b/2: interleave remaining QKV with head-0/1 attention -
        fillers = (
            make_qk_filler(1, 0, QO, QT_sb, False, "gQ2")
            + make_qk_filler(1, 1, QO, QT_sb, False, "gQ3")
            + make_qk_filler(1, 0, KO, KT_sb, True, "gK2")
            + make_qk_filler(1, 1, KO, KT_sb, True, "gK3")
            + make_qk_filler(2, 0, QO, QT_sb, False, "gQ4")
            + make_qk_filler(2, 1, QO, QT_sb, False, "gQ5")
            + make_qk_filler(2, 0, KO, KT_sb, True, "gK4")
            + make_qk_filler(2, 1, KO, KT_sb, True, "gK5")
        )
        for t8 in range(8):
            fillers += make_v_filler(t8)
        fq = iter(fillers)

        def fill(n):
            for _ in range(n):
                f = next(fq, None)
                if f:
                    f()

        ex_tiles = {}
        ex_tiles[(0, 0)] = emit_e(0, 0)
        gK1 = psA.tile([128, 512], F32, tag="mm", bufs=2, name="gK1")
        emit_qk_half(gK1, 0, 1, KO, range(CC))
        qk_copy(gK1, 1, KT_sb, 0, True)
        ex_tiles[(0, 1)] = emit_e(0, 1)
        for kc in range(2, 8):
            fill(2)
            ex_tiles[(0, kc)] = emit_e(0, kc)
        fill(6)  # finish QK groups + V0 halves
        # head 0 AVs interleaved with head-1 E and remaining QK groups
        po = begin_head(0)
        for kc in range(8):
            ex_tiles[(1, kc)] = emit_e(1, kc)
            fill(2)
            emit_av(0, kc, po, ex_tiles.pop((0, kc)))
        fill(99)  # leftovers, if any
        norm_chain(0, po)

        # the QKV-group pool is done; its banks become a third exp buffer,
        # deepening the exp pipeline so ACT paces the remaining heads
        psA.release()
        psB2 = tc.alloc_tile_pool(name="psB2", bufs=1, space="PSUM")
        e_state["psB2"] = psB2
        e_state["n"] = 0

        # heads 1..4: E two chunks ahead so the first AV of each head starts
        # after the previous head's PSUM halves have both drained
        for h in range(1, 5):
            po = begin_head(h)
            ex_tiles[(h + 1, 0)] = emit_e(h + 1, 0)
            ex_tiles[(h + 1, 1)] = emit_e(h + 1, 1)
            for kc in range(8):
                if kc + 2 < 8:
                    ex_tiles[(h + 1, kc + 2)] = emit_e(h + 1, kc + 2)
                emit_av(h, kc, po, ex_tiles.pop((h, kc)))
            norm_chain(h, po)

        # head 5: straight AVs (exps already pipelined ahead)
        po = begin_head(5)
        for kc in range(8):
            emit_av(5, kc, po, ex_tiles.pop((5, kc)))

        # release attention PSUM; proj pool takes the freed 12KB (psB's
        # last reader is exp(5,7), which completes before AV(5,7) does)
        psB2.release()
        psB.release()
        psD = ctx.enter_context(tc.tile_pool(name="psD", bufs=1, space="PSUM"))
        outp = ctx.enter_context(tc.tile_pool(name="outp", bufs=3))

        # ---- phase 3: output projection (transposed) ----------------------
        # out^T[e, tok] = sum_c Wp[c-chunk, e]^T @ OT[c-chunk, tok]: the
        # stationary operand is a Wp block reused across both token halves,
        # so only 18 LDWEIGHTS total and each hides behind 1024 moving rows
        # (the row-major orientation exposed 24 of them).  The host
        # transposes the output for free during unsharding.
        def new_pso(eb):
            return psD.tile([128, N], F32, tag="pso", bufs=3, name=f"pso_{eb}")

        def proj_c(pso, eb, cs):
            for c in cs:
                for th in range(2):
                    nc.tensor.matmul(
                        pso[:, th * 512 : (th + 1) * 512],
                        Wp_sb[:, c, eb * 128 : (eb + 1) * 128],
                        OT_sb[:, c, th * 512 : (th + 1) * 512],
                        start=(c == 0),
                        stop=(c == 2),
                    )

        def proj_finish(pso, eb):
            # each drain copy splits across DVE + ACT in parallel (ACT is
            # idle after the last exp); DMAs issue from the idle gpsimd
            # queue so the sync engine's ~0.6us/issue never paces the drain
            ot = outp.tile([128, N], F16, tag="out", bufs=3, name=f"ot_{eb}")
            nc.vector.tensor_copy(ot[0:64, :], pso[0:64, :])
            nc.scalar.copy(ot[64:128, :], pso[64:128, :])
            nc.gpsimd.dma_start(out[eb * 128 : (eb + 1) * 128, :], ot[:])
            return ot

        # Head-5 chain overlap: c0/c1 passes + junk into the freed AV bank
        # keep the PE's activity window busy while the half-split chain
        # completes; the c2 token-half passes then chase mul_a / mul_b.
        pso_t = {}
        for eb in range(3):
            pso_t[eb] = new_pso(eb)
            proj_c(pso_t[eb], eb, (0, 1))
        ou5, rsr5, off5, p35 = norm_chain(5, po, last=True)
        jt2 = psO.tile([128, 512], F32, tag="o2", bufs=2, name="jt2")
        emit_junk(5, jt2)
        rb5 = [
            psO.tile([64, 512], F32, tag="o2", bufs=2, name=f"rb5_{qc}")
            for qc in range(2)
        ]
        nc.tensor.matmul(rb5[0][:], ones64[:], rsr5[0:1, 0:512], start=True, stop=True)
        nc.tensor.matmul(rb5[1][:], ones64[:], rsr5[0:1, 512:1024], start=True, stop=True)
        for qc in range(2):
            nc.vector.tensor_mul(
                OT_sb[off5 : off5 + 64, p35, qc * 512 : (qc + 1) * 512],
                ou5[0:64, qc * 512 : (qc + 1) * 512],
                rb5[qc][:],
            )
        ots = {}
        for eb in range(3):
            proj_c(pso_t[eb], eb, (2,))
            ots[eb] = proj_finish(pso_t[eb], eb)
        for eb in range(3, 6):
            pso = new_pso(eb)
            proj_c(pso, eb, (0, 1, 2))
            ots[eb] = proj_finish(pso, eb)

        if DEBUG:
            dbg = ctx.enter_context(tc.tile_pool(name="dbg", bufs=2))
            def dump(dst, src_ap, nparts, width):
                t = dbg.tile([128, width], F32, tag="d", bufs=2)
                nc.vector.tensor_copy(t[0:nparts, :], src_ap)
                nc.sync.dma_start(dst, t[0:nparts, :])
            for hh in range(HPC):
                dump(dQT[hh * 128 : (hh + 1) * 128, :], QT_sb[:, hh, :], 128, N)
                dump(dKT[hh * 128 : (hh + 1) * 128, :], KT_sb[:, hh, :], 128, N)
            for t8 in range(8):
                dump(dV[t8 * 128 : (t8 + 1) * 128, :], V_sb[:, t8, :], 128, VW)
            for cc in range(3):
                dump(dOT[cc * 128 : (cc + 1) * 128, :], OT_sb[:, cc, :], 128, N)
            dump(dEX, dbg_hold_ex[:], 128, N)
            dump(dOU, dbg_hold_ou[:], 128, N)
            dump(dPE, dbg_hold_pe[:], 128, N)
            dump(dRB, dbg_hold_rb[:], 128, N)

    nc.compile()
    return nc


def build_in_maps(x, Wqkv, bqkv, Wproj, bproj):
    x = np.asarray(x, dtype=np.float32)
    Wqkv = np.asarray(Wqkv, dtype=np.float32)
    bqkv = np.asarray(bqkv, dtype=np.float32)
    Wproj = np.asarray(Wproj, dtype=np.float32)

    s = 1.0 / math.sqrt(EMB)
    cols = np.arange(3 * EMB).reshape(HEADS, DH, 3)  # (h, d, qkv) col index map
    in_maps = []
    for core in range(N_CORES):
        b, g = divmod(core, 2)
        hsl = slice(g * HPC, (g + 1) * HPC)
        qcols = cols[hsl, :, 0].reshape(-1)
        kcols = cols[hsl, :, 1].reshape(-1)
        vcols = cols[hsl, :, 2]  # [HPC, DH]

        IN_a = np.zeros((CC * 128, INW), np.float16)
        IN_a[:, XO : XO + N] = x[b].T
        IN_a[:, QO : QO + 384] = Wqkv[:, qcols]
        IN_a[:, KO : KO + 384] = Wqkv[:, kcols]
        # per-head bias columns (chunk-0 rows = partitions), duplicated in
        # both partition halves so either scalar-operand alignment works
        bq = bqkv[qcols].reshape(HPC, DH)
        bk = bqkv[kcols].reshape(HPC, DH)
        for j in range(HPC):
            IN_a[0:64, BO + j] = bq[j]
            IN_a[64:128, BO + j] = bq[j]
            IN_a[0:64, BO + 8 + j] = bk[j]
            IN_a[64:128, BO + 8 + j] = bk[j]

        Wv_a = np.zeros((CC * 128, HPC * 65), np.float16)
        for j in range(HPC):
            Wv_a[:, j * 65 : j * 65 + DH] = Wqkv[:, vcols[j]]

        Wp_a = (Wproj[g * 384 : (g + 1) * 384] * s).astype(np.float16)

        in_maps.append({"IN": IN_a, "Wv": Wv_a, "Wp": Wp_a})
    return in_maps


_NC_CACHE = None


def _get_program():
    global _NC_CACHE
    if _NC_CACHE is None:
        _NC_CACHE = build_program()
    return _NC_CACHE


def kernel(x, Wqkv, bqkv, Wproj, bproj, **_kwargs):
    nc = _get_program()
    in_maps = build_in_maps(x, Wqkv, bqkv, Wproj, bproj)
    res = run_bass_kernel_spmd(nc, in_maps, list(range(N_CORES))).results
    # V-bias and proj-bias fold (exact: normalized softmax rows sum to 1)
    bqkv64 = np.asarray(bqkv, dtype=np.float64)
    vcols = (np.arange(3 * EMB).reshape(HEADS, DH, 3))[:, :, 2].reshape(-1)
    vec = (
        bqkv64[vcols] @ np.asarray(Wproj, dtype=np.float64) / math.sqrt(EMB)
        + np.asarray(bproj, dtype=np.float64)
    ).astype(np.float32)
    out = np.empty((B, N, EMB), np.float32)
    for b in range(B):
        out[b] = (
            res[2 * b]["out"].astype(np.float32)
            + res[2 * b + 1]["out"].astype(np.float32)
        ).T + vec
    return out
